# revision 1
# baseline (speedup 1.0000x reference)
"""Trainium2 Bass kernel for AxialSelfAttention2d (see reference in module docstring).

Reference computation (per batch b):
    qkv = W @ x + b            (1x1 conv; W [3E, E], x [E, S, L], E = 512)
    q, k, v split; q *= Dh**-0.5; per head h: q,k,v [Dh=64, S, L]
    col:  scores[s,t|l] = q[:,s,l].k[:,t,l]; softmax over t; out_col = attn @ v
    row:  scores[l,m|s] = q[:,s,l].k[:,s,m]; softmax over m; out_row = attn @ v
    out = out_col + out_row    -> [H*Dh, S, L]

Sharding: 8 cores = 2 batches x 4 head-pairs. Each core computes 2 heads of one
batch end-to-end (no collectives); the host concatenates core outputs.

Per-core dataflow (matmul operands fp16, fp32 PSUM accumulation):
  A)  x fp32 --cast-DMA--> SBUF fp16 tiles; QKV projection with W^T stationary
      -> q2, k2, v_sl [128(2h x 64d), S*L] fp16 (+ bias, q pre-scaled on host).
  A2) v_ls = v_sl reordered to (l,s) free order (gpsimd copy);
      vT_row[h][l, s*65+{d,1}] <- DMA-transpose(v_sl[h]);
      vT_col[h][s, l*65+{d,1}] <- DMA-transpose(v_ls[h]); ones columns memset.
  B)  col attention per (l, h): scoresT[t,s] = k_l^T @ q_l (PE, K=64, two heads
      row-packed via base partitions); e = exp(scoresT) (ACT, no max-subtraction
      -- scores are ~N(0,1)); AV: out[s, 65] = e^T.T @ vT_col_l (column 64 gives
      the softmax denominator); fused DVE divide (denominator broadcast with a
      step-0 free dim) -> col_src[s, l*128+hd].
  B2) DMA-transpose col_src chunks -> dst[hd, s*128+l] (final orientation).
  C)  row attention symmetric -> row_src[l, s*128+hd]; DMA-transpose chunks;
      DVE add into dst; cast-DMA (fp16 -> fp32) to DRAM out.
"""

import numpy as np
from contextlib import ExitStack

NUM_HEADS = 8
DIM_HEAD = 64
EMBED = 512
B, S, L = 2, 128, 128
SL = S * L
N_CORES = 8
HPC = 2  # heads per core

_CACHE = {}


def build_program(nc, tc):
    import concourse.bass as bass
    import concourse.mybir as mybir

    f16 = mybir.dt.float16
    f32 = mybir.dt.float32
    AF = mybir.ActivationFunctionType
    OP = mybir.AluOpType
    AP = bass.AP

    x_d = nc.dram_tensor("x", [EMBED, S, L], f32, kind="ExternalInput")
    w_d = nc.dram_tensor("wT", [EMBED, 384], f16, kind="ExternalInput")
    b_d = nc.dram_tensor("bvec", [384], f32, kind="ExternalInput")
    out_d = nc.dram_tensor("out", [128, S, L], f32, kind="ExternalOutput")

    x_flat = x_d.ap().rearrange("c s l -> c (s l)")

    CH = 32          # slice indices per chunk
    NCH = 128 // CH  # 4

    def stage_a(qk_pool, q2, k2, v_sl):
        GW = 2048  # spatial columns per x load
        with tc.tile_pool(name="xload", bufs=2) as xpool, \
             tc.tile_pool(name="wpool", bufs=1) as wpool, \
             tc.tile_pool(name="qkvps", bufs=4, space="PSUM") as qkv_ps:
            w_sb = wpool.tile([128, 4, 384], f16, tag="w")
            nc.sync.dma_start(w_sb[:],
                              w_d.ap().rearrange("(k c) o -> c k o", k=4))
            b_sb = wpool.tile([128, 3], f32, tag="b")
            nc.sync.dma_start(b_sb[:], b_d.ap().rearrange("(m p) -> p m", p=128))
            for g in range(SL // GW):
                xt = xpool.tile([128, 4, GW], f16, tag="x")
                nc.gpsimd.dma_start(
                    xt[:],
                    x_flat[:, g * GW:(g + 1) * GW]
                        .rearrange("(k c) n -> c k n", k=4))
                for m in range(3):  # 0=q, 1=k, 2=v
                    dest = (q2, k2, v_sl)[m]
                    for sg in range(GW // 512):
                        ps = qkv_ps.tile([128, 512], f32, tag="acc")
                        for c in range(4):
                            nc.tensor.matmul(
                                ps[:],
                                w_sb[:][:, c, m * 128:(m + 1) * 128],
                                xt[:][:, c, sg * 512:(sg + 1) * 512],
                                start=(c == 0), stop=(c == 3))
                        off = g * GW + sg * 512
                        nc.vector.tensor_scalar_add(
                            dest[:][:, off:off + 512], ps[:],
                            b_sb[:][:, m:m + 1])

    def make_vt(pool, tmp_pool, tagp, src, n_outer):
        """vt[h][p, i*65 + {0..63: d, 64: 1}] <- transpose of src[h-slice].

        DMA-transpose requires a packed [p, mid, last] output (strided mid
        corrupts data on HW), so transpose into a packed tmp then let gpsimd
        restride into the 65-wide augmented layout."""
        vts = []
        for h in range(HPC):
            vt = pool.tile([128, n_outer * 65], f16, tag=f"{tagp}{h}")
            for qtr in range(n_outer // 32):
                tmp = tmp_pool.tile([128, 32, 64], f16, tag="vtmp")
                nc.sync.dma_start(
                    tmp[:],
                    src[:][h * 64:(h + 1) * 64,
                           qtr * 32 * 128:(qtr + 1) * 32 * 128],
                    transpose=True)
                o = AP(vt[:].tensor, vt[:].offset + qtr * 32 * 65,
                       [list(vt[:].ap[0]), [65, 32], [1, 64]])
                nc.gpsimd.tensor_copy(o, tmp[:])
            ones_ap = AP(vt[:].tensor, vt[:].offset + 64,
                         [list(vt[:].ap[0]), [65, n_outer], [1, 1]])
            nc.vector.memset(ones_ap, 1.0)
            vts.append(vt)
        return vts

    # ---------------- attention (direction 0 = col, 1 = row) ----------------
    def attention(direction, vt, qv, kv, dst, zero_sb):
        with ExitStack() as dctx:
            src_pool = dctx.enter_context(
                tc.tile_pool(name=f"src{direction}", bufs=2))
            sc_ps = dctx.enter_context(
                tc.tile_pool(name=f"scps{direction}", bufs=2, space="PSUM"))
            av_ps = dctx.enter_context(
                tc.tile_pool(name=f"avps{direction}", bufs=2, space="PSUM"))
            e_pool = dctx.enter_context(
                tc.tile_pool(name=f"e{direction}", bufs=4))
            den_pool = dctx.enter_context(
                tc.tile_pool(name=f"den{direction}", bufs=2))
            tr_pool = None
            if direction == 0:
                tr_pool = dctx.enter_context(tc.tile_pool(name="coltr", bufs=2))

            if direction == 0:
                def qk_slice(t, h, i):  # [64, t/s] column i, stride L
                    return t[h * 64:(h + 1) * 64, :, i]
            else:
                def qk_slice(t, h, i):  # [64, m/l] row i, contiguous
                    return t[h * 64:(h + 1) * 64, i, :]

            # PSUM-bank discipline: matmuls with different tile_positions must
            # never write the same bank (HW fault) -> per-head score banks.
            for ch in range(NCH):
                src = src_pool.tile([128, CH * 128], f16, tag="src")
                for quad in range(CH // 4):
                    i0 = ch * CH + quad * 4
                    for h in range(2):
                        sc = sc_ps.tile([128, 512], f32, tag=f"sc{h}")
                        for j in range(4):
                            nc.tensor.matmul(
                                sc[:][:, j * 128:(j + 1) * 128],
                                qk_slice(kv, h, i0 + j),
                                qk_slice(qv, h, i0 + j),
                                start=True, stop=True)
                        et = e_pool.tile([128, 512], f16, tag="e")
                        nc.scalar.activation(et[:], sc[:], AF.Exp,
                                             bias=zero_sb[:][:, 0:1])
                        av = av_ps.tile([128, 260], f32, tag="av")
                        for j in range(4):
                            nc.tensor.matmul(
                                av[:][:, j * 65:(j + 1) * 65],
                                et[:][:, j * 128:(j + 1) * 128],
                                vt[h][:][:, (i0 + j) * 65:(i0 + j + 1) * 65],
                                start=True, stop=True)
                        den = den_pool.tile([128, 4], f32, tag="den")
                        nc.vector.reciprocal(
                            den[:], AP(av[:].tensor, av[:].offset + 64,
                                       [list(av[:].ap[0]), [65, 4]]))
                        # src[s, (i0+j)*128 + h*64 + d] = av[:, j*65+d]*rden[:, j]
                        in0 = AP(av[:].tensor, av[:].offset,
                                 [list(av[:].ap[0]), [65, 4], [1, 64]])
                        in1 = AP(den[:].tensor, den[:].offset,
                                 [list(den[:].ap[0]), [1, 4], [0, 64]])
                        o = AP(src[:].tensor,
                               src[:].offset + (quad * 4) * 128 + h * 64,
                               [list(src[:].ap[0]), [128, 4], [1, 64]])
                        nc.vector.tensor_tensor(o, in0, in1, OP.mult)

                if direction == 1:
                    # row runs first: transpose lands directly in dst
                    # dst[hd, (ch*CH+sr)*128 + l'] <- src[l', sr*128+hd]
                    od = AP(dst[:].tensor, dst[:].offset + ch * CH * 128,
                            [list(dst[:].ap[0]), [128, CH], [1, 128]])
                    nc.sync.dma_start(od, src[:], transpose=True)
                else:
                    # col: transpose to trc[hd, lr*128 + s], then strided add
                    tr = tr_pool.tile([128, CH * 128], f16, tag="tr")
                    ot = AP(tr[:].tensor, tr[:].offset,
                            [list(tr[:].ap[0]), [128, CH], [1, 128]])
                    nc.sync.dma_start(ot, src[:], transpose=True)
                    # dst[hd, s*128 + (ch*CH+lr)] += trc[hd, lr*128 + s]
                    dseg = AP(dst[:].tensor, dst[:].offset + ch * CH,
                              [list(dst[:].ap[0]), [1, CH], [128, S]])
                    nc.vector.tensor_add(dseg, dseg, tr[:])

    # ---------------- top-level pool nesting (LIFO) ----------------
    import os
    stage = os.environ.get("AXIAL_DEBUG_STAGE", "full")
    with tc.tile_pool(name="qk", bufs=1) as qk_pool, \
         tc.tile_pool(name="vt", bufs=1) as vt_pool:
        q2 = qk_pool.tile([128, SL], f16, tag="q2")
        k2 = qk_pool.tile([128, SL], f16, tag="k2")
        zero_sb = qk_pool.tile([128, 1], f32, tag="z")
        nc.vector.memset(zero_sb[:], 0.0)

        with tc.tile_pool(name="vsl", bufs=1) as vsl_pool:
            v_sl = vsl_pool.tile([128, SL], f16, tag="v_sl")
            stage_a(qk_pool, q2, k2, v_sl)
            vt_row = vt_col = None
            if stage != "a":
                with tc.tile_pool(name="vtmp", bufs=2) as tmp_pool:
                    vt_row = make_vt(vt_pool, tmp_pool, "vtr", v_sl, S)
                    with tc.tile_pool(name="vls", bufs=1) as vls_pool:
                        v_ls = vls_pool.tile([128, SL], f16, tag="v_ls")
                        nc.gpsimd.tensor_copy(
                            v_ls[:].rearrange("p (l s) -> p l s", s=S),
                            v_sl[:].rearrange("p (s l) -> p l s", l=L))
                        vt_col = make_vt(vt_pool, tmp_pool, "vtc", v_ls, L)

        with tc.tile_pool(name="dstp", bufs=1) as dst_pool:
            dst = dst_pool.tile([128, SL], f16, tag="dst")  # [hd, s*128+l]
            qv = q2[:].rearrange("p (s l) -> p s l", l=L)
            kv = k2[:].rearrange("p (s l) -> p s l", l=L)
            if stage in ("row", "full"):
                attention(1, vt_row, qv, kv, dst, zero_sb)  # row: fills dst
            if stage == "full":
                attention(0, vt_col, qv, kv, dst, zero_sb)  # col: adds
            if stage in ("a", "a2"):
                nc.vector.tensor_copy(dst[:], q2[:])
            for ch in range(NCH):
                nc.gpsimd.dma_start(
                    out_d.ap()[:, ch * CH:(ch + 1) * CH, :],
                    dst[:][:, ch * CH * 128:(ch + 1) * CH * 128]
                        .rearrange("p (s l) -> p s l", l=L))


def _get_nc():
    if "nc" in _CACHE:
        return _CACHE["nc"]
    import concourse.bacc as bacc
    import concourse.tile as tile

    nc = bacc.Bacc(None, target_bir_lowering=False, debug=False,
                   num_devices=N_CORES)
    with tile.TileContext(nc) as tc:
        build_program(nc, tc)
    nc.compile()
    _CACHE["nc"] = nc
    return nc


def make_in_maps(x, W, b):
    x = np.asarray(x, dtype=np.float32)
    W = np.asarray(W, dtype=np.float32)
    b = np.asarray(b, dtype=np.float32)
    scale = np.float32(DIM_HEAD ** -0.5)
    in_maps = []
    for c in range(N_CORES):
        bb, h0 = c // 4, 2 * (c % 4)
        hd = np.arange(h0 * 64, (h0 + 2) * 64)
        sel = np.concatenate([hd, EMBED + hd, 2 * EMBED + hd])
        W_loc = W[sel, :].copy()
        b_loc = b[sel].copy()
        W_loc[:128] *= scale
        b_loc[:128] *= scale
        in_maps.append({
            "x": np.ascontiguousarray(x[bb]),
            "wT": np.ascontiguousarray(W_loc.T).astype(np.float16),
            "bvec": b_loc.astype(np.float32),
        })
    return in_maps


def assemble(results):
    out = np.empty((B, EMBED, S, L), dtype=np.float32)
    for c, r in enumerate(results):
        bb, h0 = c // 4, 2 * (c % 4)
        out[bb, h0 * 64:(h0 + 2) * 64] = r["out"]
    return out


def kernel(x, W, b):
    from concourse.bass_utils import run_bass_kernel_spmd
    nc = _get_nc()
    res = run_bass_kernel_spmd(nc, make_in_maps(x, W, b),
                               core_ids=list(range(N_CORES)))
    return assemble(res.results)



# revision 58
# speedup vs baseline: 2503.1383x; 2503.1383x over previous
"""Trainium2 Bass kernel for AxialSelfAttention2d.

Reference computation (per batch b):
    qkv = W @ x + b            (1x1 conv; W [3E, E], x [E, S, L], E = 512)
    q, k, v split; q *= Dh**-0.5; per head h: q,k,v [Dh=64, S, L]
    col:  scores[s,t|l] = q[:,s,l].k[:,t,l]; softmax over t; out_col = attn @ v
    row:  scores[l,m|s] = q[:,s,l].k[:,s,m]; softmax over m; out_row = attn @ v
    out = out_col + out_row    -> [H*Dh, S, L]

Sharding: 8 cores = 2 batches x 4 head-pairs. Each core computes 2 heads of one
batch end-to-end (no collectives); the host concatenates core outputs.

Per-core dataflow (fp16 matmul operands, fp32 PSUM accumulation). PE is the
bottleneck engine (~125us of matmul at full clock), so the program is
software-pipelined THREE stages deep per direction -- projection of block g,
QK+exp of block g-2, AV+normalize of block g-3 all in flight -- so every
cross-engine hop (DMA transpose + 900ns sem, exp latency, PSUM movers) gets
about a full iteration of slack instead of sitting on the critical path.

  Phase 1, iteration g (16 blocks of 8 s-rows) emits, interleaved:
    - x(g+2) prefetch via SWDGE cast-DMA fp32->f16 (Pool queue, ahead of
      the v_ls scatter so it is never stuck behind it).
    - vt prep for block g-1: ONE packed DMA-transpose of v_g ->
      vt[l', s, hd] (both heads, persistent 3-deep ring), v_ls [(l,s)
      order] scatter on Pool.
    - QKV projection block g (6 accumulation groups of 4 matmuls, v FIRST
      so v_g lands early); PSUM->SBUF movers with bias: q,v on ACT
      (Identity+bias), k on DVE; 2 PSUM banks ping-pong.
    - ROW attention: QK of block g-2 (K=64, both heads, head-disjoint
      PSUM banks inside one [128,1024] tile), exp on ACT; AV of block g-3
      per (slice,head) plus an N=1 ones-column matmul reusing the same
      stationary e^T to produce the softmax denominator in the same av
      bank; DVE reciprocal + fused divide-mover (packed output via
      (slice,head)-interleaved groups); DMA-transpose into dst[hd,s*128+l].
  Phase 2 (8 blocks of 16 l-slices): COL attention symmetric (vt_col from
    v_ls, prefetched 2 blocks ahead, 4-deep ring), same 3-stage pipeline
    (QK of cb, AV of cb-2), strided dst-adds on Pool by s-quarter,
    contiguous f16 out DMA per s-quarter at the end (host upcasts).
"""

import numpy as np

NUM_HEADS = 8
DIM_HEAD = 64
EMBED = 512
B, S, L = 2, 128, 128
SL = S * L
N_CORES = 8
HPC = 2  # heads per core

GW = 1024          # spatial columns per x block (8 s-rows)
NG = SL // GW      # 16
RB = GW // L       # 8 row slices per block
CB = 16            # col slices per block
NCB = L // CB      # 8
VST = 80           # vt stride: 64 d + ones@64 + pad (80*2B = 32B-aligned)

_CACHE = {}


def build_program(nc, tc):
    import concourse.bass as bass
    import concourse.mybir as mybir

    f16 = mybir.dt.float16
    f32 = mybir.dt.float32
    AF = mybir.ActivationFunctionType
    OP = mybir.AluOpType
    AP = bass.AP

    import os
    dbg = os.environ.get("AXIAL_DEBUG", "") == "1"

    x_d = nc.dram_tensor("x", [EMBED, S, L], f32, kind="ExternalInput")
    w_d = nc.dram_tensor("wT", [EMBED, 384], f16, kind="ExternalInput")
    b_d = nc.dram_tensor("bvec", [384], f32, kind="ExternalInput")
    out_d = nc.dram_tensor("out", [128, SL], f16, kind="ExternalOutput")
    if dbg:
        dbg_d = {n: nc.dram_tensor(f"dbg_{n}", [128, SL], f16,
                                   kind="ExternalOutput")
                 for n in ("q2", "k2", "vls")}
        dbg2_d = {"et0": nc.dram_tensor("dbg_et0", [128, 1024], f16,
                                        kind="ExternalOutput"),
                  "src0": nc.dram_tensor("dbg_src0", [128, RB * 128], f16,
                                         kind="ExternalOutput"),
                  "dstrow": nc.dram_tensor("dbg_dstrow", [128, SL], f16,
                                           kind="ExternalOutput")}

    x_flat = x_d.ap().rearrange("c s l -> c (s l)")

    def init_vt(pool, tagp, nsl, nbuf):
        """Persistent vt ring buffers: vt[l', slice, hd] covers BOTH heads
        (one packed transpose of the [128, nsl*128] v block per fill)."""
        bufs = []
        for p in range(nbuf):
            vt = pool.tile([128, nsl, 128], f16, tag=f"{tagp}{p}")
            bufs.append(vt)
        return bufs

    def fill_vt(vt, src_ap):
        nc.sync.dma_start(vt[:], src_ap, transpose=True)

    class AttnBlock:
        """One attention block: nsl slices starting at i0, direction given
        by qk_slice. Emission split into steps for software pipelining."""

        def __init__(self, i0, qk_slice, sc_pool, av_pool, src, vt,
                     dbg_row=False):
            self.i0, self.qk_slice = i0, qk_slice
            self.sc_pool, self.av_pool, self.src = sc_pool, av_pool, src
            self.vt = vt
            self.dbg_row = dbg_row
            self.et = {}

        def emit_qk(self, q4, qv, kv, et_pool):
            sc = self.sc_pool.tile([128, 1024], f32, tag="sc")
            for h in range(HPC):
                for j in range(4):
                    i = self.i0 + q4 * 4 + j
                    nc.tensor.matmul(
                        sc[:][:, h * 512 + j * 128:h * 512 + (j + 1) * 128],
                        self.qk_slice(kv, h, i), self.qk_slice(qv, h, i),
                        start=True, stop=True)
            et = et_pool.tile([128, 1024], f16, tag="et")
            nc.scalar.activation(et[:], sc[:], AF.Exp)
            if dbg and q4 == 0 and self.i0 == 0 and self.dbg_row:
                nc.sync.dma_start(dbg2_d["et0"].ap(), et[:])
            self.et[q4] = et

        def emit_av(self, q4, den_pool, ones_col):
            et = self.et.pop(q4)
            for t in range(2):  # av tile: 2 slices x 2 heads (+4 den cols)
                av = self.av_pool.tile([128, 260], f32, tag="av")
                for gi in range(4):
                    j, h = (t * 2 + gi // 2), gi % 2
                    jl = q4 * 4 + j  # slice index within block
                    et_g = et[:][:, h * 512 + j * 128:h * 512 + (j + 1) * 128]
                    nc.tensor.matmul(
                        av[:][:, gi * 64:(gi + 1) * 64], et_g,
                        self.vt[:][:, jl, h * 64:(h + 1) * 64],
                        start=True, stop=True)
                    nc.tensor.matmul(
                        av[:][:, 256 + gi:257 + gi], et_g, ones_col[:],
                        start=True, stop=True)
                den = den_pool.tile([128, 4], f32, tag="den")
                nc.vector.reciprocal(den[:], av[:][:, 256:260])
                in0 = AP(av[:].tensor, av[:].offset,
                         [list(av[:].ap[0]), [64, 4], [1, 64]])
                in1 = AP(den[:].tensor, den[:].offset,
                         [list(den[:].ap[0]), [1, 4], [0, 64]])
                o = AP(self.src[:].tensor,
                       self.src[:].offset + (q4 * 4 + t * 2) * 128,
                       [list(self.src[:].ap[0]), [64, 4], [1, 64]])
                nc.vector.tensor_tensor(o, in0, in1, OP.mult)

    with tc.tile_pool(name="wp", bufs=1) as wpool, \
         tc.tile_pool(name="qk", bufs=1) as qk_pool, \
         tc.tile_pool(name="et", bufs=12) as et_pool, \
         tc.tile_pool(name="den", bufs=4) as den_pool, \
         tc.tile_pool(name="vtp", bufs=1) as vt_pool, \
         tc.tile_pool(name="dstp", bufs=1) as dst_pool:
        w_sb = wpool.tile([128, 4, 384], f16, tag="w")
        nc.sync.dma_start(w_sb[:], w_d.ap().rearrange("(k c) o -> c k o", k=4))
        b_sb = wpool.tile([128, 3], f32, tag="b")
        nc.sync.dma_start(b_sb[:], b_d.ap().rearrange("(m p) -> p m", p=128))

        q2 = qk_pool.tile([128, SL], f16, tag="q2")
        k2 = qk_pool.tile([128, SL], f16, tag="k2")
        v_ls = qk_pool.tile([128, SL], f16, tag="v_ls")
        dst = dst_pool.tile([128, SL], f16, tag="dst")  # [hd, s*128+l]

        qv = q2[:].rearrange("p (s l) -> p s l", l=L)
        kv = k2[:].rearrange("p (s l) -> p s l", l=L)

        def row_slice(t, h, i):  # [64, m/l] row i, contiguous
            return t[h * 64:(h + 1) * 64, i, :]

        def col_slice(t, h, i):  # [64, t/s] column i, stride L
            return t[h * 64:(h + 1) * 64, :, i]

        ones_col = qk_pool.tile([128, 1], f16, tag="ones")
        nc.vector.memset(ones_col[:], 1.0)
        vtr = init_vt(vt_pool, "vtr", RB, 3)

        # ---------------- phase 1: projection + row attention ----------
        with tc.tile_pool(name="xp", bufs=3) as xpool, \
             tc.tile_pool(name="vg", bufs=4) as vg_pool, \
             tc.tile_pool(name="srcr", bufs=3) as srcr_pool, \
             tc.tile_pool(name="pps", bufs=2, space="PSUM") as proj_ps, \
             tc.tile_pool(name="scr", bufs=2, space="PSUM") as sc_ps, \
             tc.tile_pool(name="avr", bufs=2, space="PSUM") as av_ps:

            def load_x(g):
                xt = xpool.tile([128, 4, GW], f16, tag="x")
                with tc.high_priority():
                    nc.gpsimd.dma_start(
                        xt[:],
                        x_flat[:, g * GW:(g + 1) * GW]
                            .rearrange("(k c) n -> c k n", k=4))
                return xt

            def prep_row(pg, pv):
                """vt transpose (SP) + v_ls scatter (Pool) for block pg."""
                vt = vtr[pg % 3]
                fill_vt(vt, pv[:])
                vls_o = AP(v_ls[:].tensor, v_ls[:].offset + pg * RB,
                           [list(v_ls[:].ap[0]), [1, RB], [128, L]])
                vg_i = AP(pv[:].tensor, pv[:].offset,
                          [list(pv[:].ap[0]), [128, RB], [1, L]])
                nc.gpsimd.tensor_copy(vls_o, vg_i)
                return vt

            def make_row_blk(ag):
                src = srcr_pool.tile([128, RB * 128], f16, tag="src")
                return AttnBlock(ag * RB, row_slice, sc_ps, av_ps, src,
                                 fills.pop(ag), dbg_row=(ag == 0))

            def finish_dst(blk, g):
                if dbg and g == 0:
                    nc.sync.dma_start(dbg2_d["src0"].ap(), blk.src[:])
                od = AP(dst[:].tensor, dst[:].offset + g * RB * 128,
                        [list(dst[:].ap[0]), [128, RB], [1, 128]])
                nc.sync.dma_start(od, blk.src[:], transpose=True)

            xts = {g: load_x(g) for g in range(2)}
            fills = {}
            blks = {}
            prev_vg = None       # (g, v_g) with projection emitted
            for g in range(NG):
                if g + 2 < NG:
                    xts[g + 2] = load_x(g + 2)
                if prev_vg is not None:
                    fills[prev_vg[0]] = prep_row(*prev_vg)
                # QK+exp for block g-2; AV+div for block g-3 (3-stage pipe)
                blkq = make_row_blk(g - 2) if g >= 2 else None
                blka = blks.pop(g - 3, None)

                xt = xts.pop(g)
                v_g = vg_pool.tile([128, GW], f16, tag="vg")

                def proj_group(m, sg):
                    ps = proj_ps.tile([128, 512], f32, tag="acc")
                    for c in range(4):
                        nc.tensor.matmul(
                            ps[:],
                            w_sb[:][:, c, m * 128:(m + 1) * 128],
                            xt[:][:, c, sg * 512:(sg + 1) * 512],
                            start=(c == 0), stop=(c == 3))
                    off = g * GW + sg * 512
                    bias = b_sb[:][:, m:m + 1]
                    if m == 0:
                        nc.scalar.activation(q2[:][:, off:off + 512], ps[:],
                                             AF.Identity, bias=bias)
                    elif m == 1:
                        nc.vector.tensor_scalar_add(
                            k2[:][:, off:off + 512], ps[:], bias)
                    else:
                        nc.scalar.activation(
                            v_g[:][:, sg * 512:(sg + 1) * 512], ps[:],
                            AF.Identity, bias=bias)

                # interleave: proj groups of g (v first, so v_g lands early
                # and the vt fill has a full iteration of slack) with QK of
                # block g-2 and AV of block g-3
                proj_group(2, 0)
                proj_group(2, 1)
                if blkq is not None:
                    blkq.emit_qk(0, qv, kv, et_pool)
                proj_group(1, 0)
                proj_group(1, 1)
                if blkq is not None:
                    blkq.emit_qk(1, qv, kv, et_pool)
                proj_group(0, 0)
                if blka is not None:
                    blka.emit_av(0, den_pool, ones_col)
                proj_group(0, 1)
                if blka is not None:
                    blka.emit_av(1, den_pool, ones_col)
                    finish_dst(blka, g - 3)
                if blkq is not None:
                    blks[g - 2] = blkq
                prev_vg = (g, v_g)

            # epilogue: drain the pipeline (QK for last 2, AV for last 3)
            fills[prev_vg[0]] = prep_row(*prev_vg)
            for ag in (NG - 2, NG - 1):
                blkq = make_row_blk(ag)
                for q4 in range(2):
                    blkq.emit_qk(q4, qv, kv, et_pool)
                blks[ag] = blkq
                blka = blks.pop(ag - 1)
                for q4 in range(2):
                    blka.emit_av(q4, den_pool, ones_col)
                finish_dst(blka, ag - 1)
            blka = blks.pop(NG - 1)
            for q4 in range(2):
                blka.emit_av(q4, den_pool, ones_col)
            finish_dst(blka, NG - 1)

        if dbg:
            nc.sync.dma_start(dbg2_d["dstrow"].ap(), dst[:])

        # ---------------- phase 2: col attention + output --------------
        with tc.tile_pool(name="vtcp", bufs=1) as vtc_pool, \
             tc.tile_pool(name="srcc", bufs=2) as srcc_pool, \
             tc.tile_pool(name="trp", bufs=2) as tr_pool, \
             tc.tile_pool(name="scc", bufs=3, space="PSUM") as sc_ps2, \
             tc.tile_pool(name="avc", bufs=2, space="PSUM") as av_ps2:
            vtc = init_vt(vtc_pool, "vtc", CB, 4)

            def prep_col(cb):
                vt = vtc[cb % 4]
                fill_vt(vt, v_ls[:][:, cb * CB * 128:(cb + 1) * CB * 128])
                return vt

            def make_col_blk(cb, vt):
                src = srcc_pool.tile([128, CB * 128], f16, tag="src")
                return AttnBlock(cb * CB, col_slice, sc_ps2, av_ps2, src, vt)

            def finish_col(blk, cb):
                tr = tr_pool.tile([128, CB * 128], f16, tag="tr")
                ot = AP(tr[:].tensor, tr[:].offset,
                        [list(tr[:].ap[0]), [128, CB], [1, 128]])
                nc.sync.dma_start(ot, blk.src[:], transpose=True)
                # dst[hd, s*128 + cb*CB + lr] += tr[hd, lr*128 + s], by
                # s-quarter, on Pool (DVE is the phase-2 bottleneck).
                for sq in range(4):
                    dseg = AP(dst[:].tensor,
                              dst[:].offset + sq * 32 * 128 + cb * CB,
                              [list(dst[:].ap[0]), [1, CB], [128, 32]])
                    tseg = AP(tr[:].tensor, tr[:].offset + sq * 32,
                              [list(tr[:].ap[0]), [128, CB], [1, 32]])
                    nc.gpsimd.tensor_add(dseg, dseg, tseg)
                    if cb == NCB - 1:
                        nc.sync.dma_start(
                            out_d.ap()[:, sq * 4096:(sq + 1) * 4096],
                            dst[:][:, sq * 4096:(sq + 1) * 4096])

            cfills = {0: prep_col(0), 1: prep_col(1)}
            cblks = {}
            for cb in range(NCB):
                blkq = make_col_blk(cb, cfills.pop(cb))
                blka = cblks.pop(cb - 2, None)
                for q4 in range(4):
                    blkq.emit_qk(q4, qv, kv, et_pool)
                    if blka is not None:
                        blka.emit_av(q4, den_pool, ones_col)
                if blka is not None:
                    finish_col(blka, cb - 2)
                # prep after the AV reads of cb-2 (same vt ring parity)
                if cb + 2 < NCB:
                    cfills[cb + 2] = prep_col(cb + 2)
                cblks[cb] = blkq
            for cb in (NCB - 2, NCB - 1):
                blka = cblks.pop(cb)
                for q4 in range(4):
                    blka.emit_av(q4, den_pool, ones_col)
                finish_col(blka, cb)

        if dbg:
            for name, t in (("q2", q2), ("k2", k2), ("vls", v_ls)):
                nc.sync.dma_start(dbg_d[name].ap(), t[:])


def _get_nc():
    if "nc" in _CACHE:
        return _CACHE["nc"]
    import concourse.bacc as bacc
    import concourse.tile as tile

    nc = bacc.Bacc(None, target_bir_lowering=False, debug=False,
                   num_devices=N_CORES)
    with tile.TileContext(nc) as tc:
        build_program(nc, tc)
    nc.compile()
    _CACHE["nc"] = nc
    return nc


def make_in_maps(x, W, b):
    x = np.asarray(x, dtype=np.float32)
    W = np.asarray(W, dtype=np.float32)
    b = np.asarray(b, dtype=np.float32)
    scale = np.float32(DIM_HEAD ** -0.5)
    in_maps = []
    for c in range(N_CORES):
        bb, h0 = c // 4, 2 * (c % 4)
        hd = np.arange(h0 * 64, (h0 + 2) * 64)
        sel = np.concatenate([hd, EMBED + hd, 2 * EMBED + hd])
        W_loc = W[sel, :].copy()
        b_loc = b[sel].copy()
        W_loc[:128] *= scale
        b_loc[:128] *= scale
        in_maps.append({
            "x": np.ascontiguousarray(x[bb]),
            "wT": np.ascontiguousarray(W_loc.T).astype(np.float16),
            "bvec": b_loc.astype(np.float32),
        })
    return in_maps


def assemble(results):
    out = np.empty((B, EMBED, S, L), dtype=np.float32)
    for c, r in enumerate(results):
        bb, h0 = c // 4, 2 * (c % 4)
        out[bb, h0 * 64:(h0 + 2) * 64] = r["out"].reshape(128, S, L)
    return out


def kernel(x, W, b):
    from concourse.bass_utils import run_bass_kernel_spmd
    nc = _get_nc()
    res = run_bass_kernel_spmd(nc, make_in_maps(x, W, b),
                               core_ids=list(range(N_CORES)))
    return assemble(res.results)
